# revision 36
# baseline (speedup 1.0000x reference)
"""FAISS-anchor kernel layer on 8 Trainium2 NeuronCores (Bass/Tile).

Problem (per full input):
    x [8,8192,3], Key [1024,3], init_mat/Value [1024,256],
    w1 [3,1024], b1 [1024], w2 [1024,256], b2 [256]
    idx = argmin_a ||x - Key_a||^2           (exact 1-NN, first-tie)
    out = gelu((x - Key[idx]) @ w1 + b1) @ w2 + b2 + (init_mat + Value)[idx]

Sharding: pure data-parallel — core c takes batch element c (8192 tokens).
All tables (Key-derived features, V-table, MLP weights) are replicated.

Device pipeline, software-pipelined with a 2-chunk skew so the PE stream
never head-of-line blocks on the gather chain. Per tile-step j of
iteration i (chunks cs=i, ch=i-1, cw=i-2):
    PE:   s(cs,j)  = -d2+|x|^2 for 1024 anchors (two row-group-packed
          f32r matmuls, K=5 feature folding)
    DVE:  max8 -> top-8; max_index -> anchor idx (first-tie == argmin)
    DMA:  indirect gather of fused row [V+init+b2 | 2*Key] per token
    GPS:  rl = 2x - 2*Key[idx]  (bf16 out)
    PE:   transpose rl -> [4,tok] bf16 at row 0 and 32 (iteration top);
          h-step: 2-way row-group-packed bf16 matmuls (K=4), the two
          groups compute different hc -> different PSUM banks
    ACT:  gelu(h + b1) -> ht bf16
    PE:   w2-step(cw,q=j): out = ht_q @ w2 (bf16, K=8x128 accum)
    ACT+GPS: PSUM->SBUF copy, += gathered V row; DMA out.

Host: packs layouts, runs 8 cores via run_bass_kernel_spmd, re-assembles,
and re-resolves near-tie tokens (top-2 gap below tau) with exact fp32
reference arithmetic so matmul rounding cannot flip the argmin.
"""

import numpy as np

B, N, A, D_IN, D_OUT = 8, 8192, 1024, 3, 256
H = 4 * D_OUT
P = 128
NT = N // P            # 64 token tiles per core
TPC = 8                # tiles per chunk
NCHUNK = NT // TPC     # 8 chunks
VT_W = 272             # gather-table row width (256 V + 3 key + pad), 1088B
N_CORES = 8
HC = H // P            # 8 H-chunks

_PROGRAM = None  # (nc, input_names)


def _build_program():
    import concourse.bass as bass
    import concourse.mybir as mybir
    import concourse.tile as tile
    from concourse import bacc

    f32 = mybir.dt.float32
    f32r = mybir.dt.float32r
    bf16 = mybir.dt.bfloat16
    u32 = mybir.dt.uint32

    # Bacc (not raw Bass): its compile() splits multi-sem waits and moves
    # matmul waits onto ldweights — TRN2 allows at most 1 wait per instr.
    nc = bacc.Bacc("TRN2", target_bir_lowering=False, debug=False)

    # DRAM I/O
    xh_d = nc.dram_tensor("xh", [P, N], f32r, kind="ExternalInput").ap()
    kh_d = nc.dram_tensor("kh", [P, A], f32r, kind="ExternalInput").ap()
    x2_d = nc.dram_tensor("x2", [P, NT * 4], f32, kind="ExternalInput").ap()
    w1h_d = nc.dram_tensor("w1h", [P, H], bf16, kind="ExternalInput").ap()
    b1p_d = nc.dram_tensor("b1p", [P, HC], f32, kind="ExternalInput").ap()
    w2b_d = nc.dram_tensor("w2b", [P, HC * D_OUT], bf16, kind="ExternalInput").ap()
    vt_d = nc.dram_tensor("vt", [A, VT_W], f32, kind="ExternalInput").ap()
    id_d = nc.dram_tensor("ident", [P, P], bf16, kind="ExternalInput").ap()

    out_d = nc.dram_tensor("outp", [N, D_OUT], f32, kind="ExternalOutput").ap()
    m8_d = nc.dram_tensor("m8o", [P, NT * 8], f32, kind="ExternalOutput").ap()
    idx_d = nc.dram_tensor("idxo", [P, NT * 8], u32, kind="ExternalOutput").ap()

    with tile.TileContext(nc) as tc:
        with (
            tc.tile_pool(name="const", bufs=1) as cpool,
            tc.tile_pool(name="xh", bufs=2) as xhpool,
            tc.tile_pool(name="vg", bufs=4) as vgpool,
            tc.tile_pool(name="rlt", bufs=2) as rltpool,
            tc.tile_pool(name="ht", bufs=2) as htpool,
            tc.tile_pool(name="m8", bufs=1) as m8pool,
            tc.tile_pool(name="idx", bufs=2) as idxpool,
            tc.tile_pool(name="ob", bufs=4) as obpool,
            tc.tile_pool(name="rl", bufs=16) as rlpool,
            tc.tile_pool(name="spsum", bufs=2, space="PSUM") as spsum,
            tc.tile_pool(name="rpsum", bufs=1, space="PSUM") as rpsum,
            tc.tile_pool(name="hpsum", bufs=2, space="PSUM") as hpsum,
            tc.tile_pool(name="opsum", bufs=1, space="PSUM") as opsum,
        ):
            # Resident constants — the s-chain deps (kh, then the chunk-0
            # xh below) are loaded first so the pipeline fills sooner; the
            # MLP constants follow (not needed until iteration 1).
            kh_t = cpool.tile([P, A], f32r)
            nc.sync.dma_start(out=kh_t[:], in_=kh_d[:])
            xh0_t = None  # placeholder, set right after pools
            x2_t = cpool.tile([P, NT, 4], f32)
            w1h_t = cpool.tile([P, H], bf16)
            b1p_t = cpool.tile([P, HC], f32)
            w2b_t = cpool.tile([P, HC * D_OUT], bf16)
            id_t = cpool.tile([P, P], bf16)
            m8_t = m8pool.tile([P, NT * 8], f32)
            # single-bank double-buffered w2 output accumulator
            o2_t = opsum.tile([P, 2, D_OUT], f32, name="o2")

            # PE warmup: back-to-back matmuls during pipeline fill push
            # the HAM clock gate to 8/8 (2.4 GHz); filler matmuls in the
            # first two iterations (below) keep it there until the real
            # w2 stream takes over.
            wu_t = cpool.tile([P, 512], bf16, name="warmup")
            nc.vector.memset(wu_t[:], 0.0)
            for r in range(24):
                nc.tensor.matmul(
                    out=o2_t[:, 0, :],
                    lhsT=wu_t[:, 0:P],
                    rhs=wu_t[:, 0:D_OUT],
                    start=True,
                    stop=True,
                )

            # Pipeline state carried across iterations
            pend_out = []      # deferred (+vg, DMA-out) work, 1 step behind
            xh_tiles = {}
            vg_tiles = {}      # chunk -> [8 tiles]
            rl_tiles = {}      # chunk -> [8 tiles]
            rlts_tiles = {}    # chunk -> [2 half tiles, SBUF]
            ht_tiles = {}      # chunk -> [2][HC] tiles
            idx_tiles = {}

            # Prefetch chunk 0 lhsT features, then the remaining consts
            xh_tiles[0] = xhpool.tile([P, TPC * P], f32r, tag="xh", name="xh0")
            nc.sync.dma_start(
                out=xh_tiles[0][:], in_=xh_d[:, 0 : TPC * P]
            )
            nc.sync.dma_start(out=x2_t[:], in_=x2_d[:])
            nc.sync.dma_start(out=w1h_t[:], in_=w1h_d[:])
            nc.sync.dma_start(out=b1p_t[:], in_=b1p_d[:])
            nc.sync.dma_start(out=w2b_t[:], in_=w2b_d[:])
            nc.sync.dma_start(out=id_t[:], in_=id_d[:])

            for i in range(NCHUNK + 2):
                cs, ch, cw = i, i - 1, i - 2

                if cs < NCHUNK:
                    idx_tiles[cs] = idxpool.tile([P, TPC, 8], u32, tag="idx", name=f"idxt{cs}")
                    vg_tiles[cs] = []
                    rl_tiles[cs] = []
                if 0 <= ch < NCHUNK:
                    ht_tiles[ch] = [
                        [
                            htpool.tile([P, 4, P], bf16,
                                        tag=f"ht{h}_{hc}",
                                        name=f"ht{ch}_{h}_{hc}")
                            for hc in range(HC)
                        ]
                        for h in range(2)
                    ]
                    # transpose rl -> rlt_ps [36, 512] (rows 0-3 and a
                    # replica at 32-35 for the second h row group), then one
                    # ACT copy per row-group to SBUF. half0 at iteration
                    # top; half1 after step 0 (gives the single rpsum bank
                    # time to drain through the ACT copies).
                    rlts_tiles[ch] = []

                    def emit_half_transpose(ch_, h_):
                        rlt_ps = rpsum.tile([4, 512], bf16, tag="rlt",
                                            name=f"rltps{ch_}_{h_}")
                        for q in range(4):
                            nc.tensor.transpose(
                                out=rlt_ps[:, q * P : (q + 1) * P],
                                in_=rl_tiles[ch_][h_ * 4 + q][:],
                                identity=id_t[:],
                            )
                        rlts = rltpool.tile([4, 512], bf16,
                                            tag=f"rlts{h_}",
                                            name=f"rlts{ch_}_{h_}")
                        nc.scalar.activation(
                            out=rlts[:],
                            in_=rlt_ps[:],
                            func=mybir.ActivationFunctionType.Copy,
                        )
                        rlts_tiles[ch_].append(rlts)

                    emit_half_transpose(ch, 0)

                for j in range(TPC):
                    # drain deferred output adds (their ACT copy is long
                    # done, so the GPS add never stalls the gather chain)
                    while len(pend_out) > (1 if cs < NCHUNK else 0):
                        cw_, j_, t_, ob_ = pend_out.pop(0)
                        nc.gpsimd.tensor_tensor(
                            out=ob_[:],
                            in0=ob_[:],
                            in1=vg_tiles[cw_][j_][:, 0:D_OUT],
                            op=mybir.AluOpType.add,
                        )
                        nc.sync.dma_start(
                            out=out_d[t_ * P : (t_ + 1) * P, :], in_=ob_[:]
                        )
                    if cs < NCHUNK:
                        t = cs * TPC + j
                        xh_t = xh_tiles[cs]
                        s_ps = spsum.tile([P, A], f32, tag="s")
                        for g in range(2):  # two 512-anchor halves, packed
                            nc.tensor.matmul(
                                out=s_ps[:, g * 512 : (g + 1) * 512],
                                lhsT=xh_t[
                                    32 * g : 32 * g + 5, j * P : (j + 1) * P
                                ],
                                rhs=kh_t[
                                    32 * g : 32 * g + 5, g * 512 : (g + 1) * 512
                                ],
                                start=True,
                                stop=True,
                                tile_position=(32 * g, 0),
                            )
                        m8 = m8_t[:, t * 8 : (t + 1) * 8]
                        nc.vector.max(m8, s_ps[:])
                        nc.vector.max_index(idx_tiles[cs][:, j, :], m8, s_ps[:])

                        if j == 0 and cs + 1 < NCHUNK:
                            xh_tiles[cs + 1] = xhpool.tile(
                                [P, TPC * P], f32r, tag="xh",
                                name=f"xh{cs + 1}",
                            )
                            nc.sync.dma_start(
                                out=xh_tiles[cs + 1][:],
                                in_=xh_d[
                                    :, (cs + 1) * TPC * P : (cs + 2) * TPC * P
                                ],
                            )

                        # Gather fused table row per token. NB: one offset per
                        # partition ([P,1]) per call into an offset-0 [P, W]
                        # dest tile — multi-index offsets and non-zero dest
                        # offsets are mishandled by the real SWDGE.
                        vg_j = vgpool.tile([P, VT_W], f32, tag=f"vg{j}")
                        nc.gpsimd.indirect_dma_start(
                            out=vg_j[:],
                            out_offset=None,
                            in_=vt_d[:],
                            in_offset=bass.IndirectOffsetOnAxis(
                                ap=idx_tiles[cs][:, j, 0:1], axis=0
                            ),
                        )
                        vg_tiles[cs].append(vg_j)

                        rl_j = rlpool.tile([P, 4], bf16, tag="rl",
                                           name=f"rl{cs}_{j}")
                        nc.gpsimd.tensor_tensor(
                            out=rl_j[:],
                            in0=x2_t[:, t, :],
                            in1=vg_j[:, D_OUT : D_OUT + 4],
                            op=mybir.AluOpType.subtract,
                        )
                        rl_tiles[cs].append(rl_j)

                        if j == TPC - 1:
                            nc.sync.dma_start(
                                out=idx_d[:, cs * TPC * 8 : (cs + 1) * TPC * 8],
                                in_=idx_tiles[cs][:],
                            )
                            nc.sync.dma_start(
                                out=m8_d[:, cs * TPC * 8 : (cs + 1) * TPC * 8],
                                in_=m8_t[:, cs * TPC * 8 : (cs + 1) * TPC * 8],
                            )

                    if 0 <= ch < NCHUNK and j == 1:
                        emit_half_transpose(ch, 1)

                    if 0 <= ch < NCHUNK:
                        # h-step: pair m=j%4 of half j//4 — two row groups
                        # compute hc=2m and hc=2m+1 concurrently into two
                        # different PSUM banks.
                        h, m = j // 4, j % 4
                        h_ps = [
                            hpsum.tile([P, 512], f32, tag="h",
                                       name=f"hps{ch}_{j}_{rg}")
                            for rg in range(2)
                        ]
                        for rg in range(2):
                            hc = 2 * m + rg
                            nc.tensor.matmul(
                                out=h_ps[rg][:],
                                lhsT=w1h_t[0:4, hc * P : (hc + 1) * P],
                                rhs=rlts_tiles[ch][h][:],
                                start=True,
                                stop=True,
                            )
                        for rg in range(2):
                            hc = 2 * m + rg
                            nc.scalar.activation(
                                out=ht_tiles[ch][h][hc][:],
                                in_=h_ps[rg][:],
                                func=mybir.ActivationFunctionType.Gelu,
                                bias=b1p_t[:, hc : hc + 1],
                            )

                    # w2 at 1.5-chunk skew: chunk c's half0 is consumed in
                    # steps 4-7 of iteration c+1 (its GELUs finished during
                    # steps 0-3); half1 in steps 0-3 of iteration c+2.
                    wt = None
                    if j < 4:
                        if cw >= 0:
                            wt = (cw, 1, j)
                    elif 0 <= ch < NCHUNK:
                        wt = (ch, 0, j - 4)
                    if wt is not None:
                        wc, wh, wq = wt
                        t = wc * TPC + wh * 4 + wq
                        o_ps = o2_t[:, j % 2, :]
                        for hc in range(HC):
                            nc.tensor.matmul(
                                out=o_ps,
                                lhsT=ht_tiles[wc][wh][hc][:, wq, :],
                                rhs=w2b_t[:, hc * D_OUT : (hc + 1) * D_OUT],
                                start=(hc == 0),
                                stop=(hc == HC - 1),
                            )
                        ob = obpool.tile([P, D_OUT], f32, tag="ob")
                        nc.scalar.activation(
                            out=ob[:],
                            in_=o_ps,
                            func=mybir.ActivationFunctionType.Copy,
                        )
                        pend_out.append((wc, wh * 4 + wq, t, ob))

                # drop references no longer needed
                if cw - 1 >= 0:
                    for dct in (vg_tiles, rl_tiles, rlts_tiles, ht_tiles,
                                idx_tiles, xh_tiles):
                        dct.pop(cw - 1, None)

            for cw_, j_, t_, ob_ in pend_out:
                nc.gpsimd.tensor_tensor(
                    out=ob_[:],
                    in0=ob_[:],
                    in1=vg_tiles[cw_][j_][:, 0:D_OUT],
                    op=mybir.AluOpType.add,
                )
                nc.sync.dma_start(
                    out=out_d[t_ * P : (t_ + 1) * P, :], in_=ob_[:]
                )
            pend_out.clear()

    nc.compile()
    names = ["xh", "kh", "x2", "w1h", "b1p", "w2b", "vt", "ident"]
    return nc, names


def _get_program():
    global _PROGRAM
    if _PROGRAM is None:
        _PROGRAM = _build_program()
    return _PROGRAM


def _host_pack(x, Key, init_mat, Value, w1, b1, w2, b2):
    """Build per-core input dicts (host-side layout packing)."""
    import ml_dtypes

    f = np.float32
    bf = ml_dtypes.bfloat16
    Key = np.asarray(Key, f)
    x = np.asarray(x, f)
    k2 = np.sum(Key * Key, axis=1)  # [A]

    # khat rows: [k0,k1,k2,1,|k|^2]; s = 2x.k - |x|^2 - |k|^2 = -d2
    kh = np.zeros((P, A), f)
    kf = np.concatenate([Key, np.ones((A, 1), f), k2[:, None]], axis=1)  # [A,5]
    for g in range(2):
        kh[32 * g : 32 * g + 5, :] = kf.T

    # w1h rows 32g..32g+2 = 0.5*w1 (replicated for 4 row groups), row 3 = 0
    w1h = np.zeros((P, H), f)
    for g in range(4):
        w1h[32 * g : 32 * g + 3, :] = 0.5 * np.asarray(w1, f)
    w1h = w1h.astype(bf)

    b1p = np.asarray(b1, f).reshape(H // P, P).T.copy()  # [128, 8]
    w2b = (
        np.asarray(w2, f)
        .reshape(H // P, P, D_OUT)
        .transpose(1, 0, 2)
        .reshape(P, (H // P) * D_OUT)
        .astype(bf)
    )
    vt = np.zeros((A, VT_W), f)
    vt[:, :D_OUT] = np.asarray(init_mat, f) + np.asarray(Value, f) + np.asarray(b2, f)
    vt[:, D_OUT : D_OUT + 3] = 2.0 * Key
    ident = np.eye(P, dtype=f).astype(bf)

    in_maps = []
    for c in range(N_CORES):
        xc = x[c]  # [N, 3]
        x2sq = np.sum(xc * xc, axis=1)  # [N]
        # xhat features [N, 5]: [2x, -|x|^2, -1]
        xf = np.concatenate(
            [2.0 * xc, -x2sq[:, None], -np.ones((N, 1), f)], axis=1
        ).astype(f)
        # packed lhsT [128, N]: tile t at cols t*128..; features at rows 0-4
        # plus a replica at rows 32-36 for the second row-group.
        xh = np.zeros((P, N), f)
        xf_t = xf.reshape(NT, P, 5).transpose(2, 0, 1).reshape(5, N)  # [5, NT*P]
        xh[0:5, :] = xf_t
        xh[32:37, :] = xf_t

        x2q = np.zeros((N, 4), f)
        x2q[:, :3] = 2.0 * xc
        x2 = x2q.reshape(NT, P, 4).transpose(1, 0, 2).reshape(P, NT * 4).copy()

        in_maps.append(
            {
                "xh": xh,
                "kh": kh,
                "x2": x2,
                "w1h": w1h,
                "b1p": b1p,
                "w2b": w2b,
                "vt": vt,
                "ident": ident,
            }
        )
    return in_maps


def _erf(z):
    # Abramowitz-Stegun is not enough; use the exact erf from scipy if
    # present, else jax (available wherever the bass stack runs).
    try:
        from scipy.special import erf

        return erf(z)
    except ImportError:
        import jax

        with jax.default_device(jax.devices("cpu")[0]):
            return np.asarray(jax.scipy.special.erf(np.asarray(z, np.float32)))


def _refine(out, m8o, idxo, x, Key, init_mat, Value, w1, b1, w2, b2, tau=0.03):
    """Re-resolve tokens whose top-2 score gap is within tau (near-ties):
    recompute their argmin + output row in exact fp32 reference arithmetic."""
    f = np.float32
    Key = np.asarray(Key, f)
    V = np.asarray(init_mat, f) + np.asarray(Value, f)
    k2 = np.sum(Key * Key, axis=1)
    n_fixed = 0
    for c in range(out.shape[0]):
        m8 = m8o[c]  # [128, NT*8]
        m0 = m8[:, 0::8]  # [128, NT]
        m1 = m8[:, 1::8]
        gap = m0 - m1  # s-space gap == d2 second - d2 min
        dev_idx = idxo[c][:, 0::8].astype(np.int64)  # [128, NT]
        scale = 1.0 + np.abs(m0)
        flag = gap < tau * scale  # [128, NT]
        ps, ts = np.nonzero(flag)
        if ps.size == 0:
            continue
        toks = ts * P + ps
        xc = np.asarray(x[c], f)[toks]  # [F, 3]
        d2 = -2.0 * (xc @ Key.T) + k2[None, :]  # reference formula, fp32
        amin = np.argmin(d2, axis=1)
        mism = amin != dev_idx[ps, ts]
        if not np.any(mism):
            continue
        toks = toks[mism]
        amin = amin[mism]
        xe = np.asarray(x[c], f)[toks]
        rl = xe - Key[amin]
        pre = (rl @ np.asarray(w1, f) + np.asarray(b1, f)).astype(f)
        h = (0.5 * pre * (1.0 + _erf(pre / np.sqrt(f(2.0))))).astype(f)
        row = (h @ np.asarray(w2, f) + np.asarray(b2, f) + V[amin]).astype(f)
        out[c, toks, :] = row
        n_fixed += toks.size
    return n_fixed


def kernel(**inputs):
    from concourse.bass_utils import run_bass_kernel_spmd

    nc, names = _get_program()
    in_maps = _host_pack(**inputs)
    res = run_bass_kernel_spmd(nc, in_maps, core_ids=list(range(N_CORES)))

    out = np.zeros((B, N, D_OUT), np.float32)
    m8o = np.zeros((B, P, NT * 8), np.float32)
    idxo = np.zeros((B, P, NT * 8), np.uint32)
    for c in range(N_CORES):
        r = res.results[c]
        out[c] = r["outp"]
        m8o[c] = r["m8o"]
        idxo[c] = r["idxo"]

    _refine(out, m8o, idxo, **inputs)
    return out


if __name__ == "__main__":
    # smoke: build only
    _get_program()
    print("program built")


# revision 37
# speedup vs baseline: 1.0109x; 1.0109x over previous
"""FAISS-anchor kernel layer on 8 Trainium2 NeuronCores (Bass/Tile).

Problem (per full input):
    x [8,8192,3], Key [1024,3], init_mat/Value [1024,256],
    w1 [3,1024], b1 [1024], w2 [1024,256], b2 [256]
    idx = argmin_a ||x - Key_a||^2           (exact 1-NN, first-tie)
    out = gelu((x - Key[idx]) @ w1 + b1) @ w2 + b2 + (init_mat + Value)[idx]

Sharding: pure data-parallel — core c takes batch element c (8192 tokens).
All tables (Key-derived features, V-table, MLP weights) are replicated.

Device pipeline, software-pipelined with a 2-chunk skew so the PE stream
never head-of-line blocks on the gather chain. Per tile-step j of
iteration i (chunks cs=i, ch=i-1, cw=i-2):
    PE:   s(cs,j)  = -d2+|x|^2 for 1024 anchors (two row-group-packed
          f32r matmuls, K=5 feature folding)
    DVE:  max8 -> top-8; max_index -> anchor idx (first-tie == argmin)
    DMA:  indirect gather of fused row [V+init+b2 | 2*Key] per token
    GPS:  rl = 2x - 2*Key[idx]  (bf16 out)
    PE:   transpose rl -> [4,tok] bf16 at row 0 and 32 (iteration top);
          h-step: 2-way row-group-packed bf16 matmuls (K=4), the two
          groups compute different hc -> different PSUM banks
    ACT:  gelu(h + b1) -> ht bf16
    PE:   w2-step(cw,q=j): out = ht_q @ w2 (bf16, K=8x128 accum)
    ACT+GPS: PSUM->SBUF copy, += gathered V row; DMA out.

Host: packs layouts, runs 8 cores via run_bass_kernel_spmd, re-assembles,
and re-resolves near-tie tokens (top-2 gap below tau) with exact fp32
reference arithmetic so matmul rounding cannot flip the argmin.
"""

import numpy as np

B, N, A, D_IN, D_OUT = 8, 8192, 1024, 3, 256
H = 4 * D_OUT
P = 128
NT = N // P            # 64 token tiles per core
TPC = 8                # tiles per chunk
NCHUNK = NT // TPC     # 8 chunks
VT_W = 272             # gather-table row width (256 V + 3 key + pad), 1088B
N_CORES = 8
HC = H // P            # 8 H-chunks

_PROGRAM = None  # (nc, input_names)


def _build_program():
    import concourse.bass as bass
    import concourse.mybir as mybir
    import concourse.tile as tile
    from concourse import bacc

    f32 = mybir.dt.float32
    f32r = mybir.dt.float32r
    bf16 = mybir.dt.bfloat16
    u32 = mybir.dt.uint32

    # Bacc (not raw Bass): its compile() splits multi-sem waits and moves
    # matmul waits onto ldweights — TRN2 allows at most 1 wait per instr.
    nc = bacc.Bacc("TRN2", target_bir_lowering=False, debug=False)

    # DRAM I/O
    xh_d = nc.dram_tensor("xh", [P, N], f32r, kind="ExternalInput").ap()
    kh_d = nc.dram_tensor("kh", [P, A], f32r, kind="ExternalInput").ap()
    x2_d = nc.dram_tensor("x2", [P, NT * 4], f32, kind="ExternalInput").ap()
    w1h_d = nc.dram_tensor("w1h", [P, H], bf16, kind="ExternalInput").ap()
    b1p_d = nc.dram_tensor("b1p", [P, HC], f32, kind="ExternalInput").ap()
    w2b_d = nc.dram_tensor("w2b", [P, HC * D_OUT], bf16, kind="ExternalInput").ap()
    vt_d = nc.dram_tensor("vt", [A, VT_W], f32, kind="ExternalInput").ap()
    id_d = nc.dram_tensor("ident", [P, P], bf16, kind="ExternalInput").ap()

    out_d = nc.dram_tensor("outp", [N, D_OUT], f32, kind="ExternalOutput").ap()
    m8_d = nc.dram_tensor("m8o", [P, NT * 8], f32, kind="ExternalOutput").ap()
    idx_d = nc.dram_tensor("idxo", [P, NT * 8], u32, kind="ExternalOutput").ap()

    with tile.TileContext(nc) as tc:
        with (
            tc.tile_pool(name="const", bufs=1) as cpool,
            tc.tile_pool(name="xh", bufs=2) as xhpool,
            tc.tile_pool(name="vg", bufs=4) as vgpool,
            tc.tile_pool(name="rlt", bufs=2) as rltpool,
            tc.tile_pool(name="ht", bufs=2) as htpool,
            tc.tile_pool(name="m8", bufs=1) as m8pool,
            tc.tile_pool(name="idx", bufs=2) as idxpool,
            tc.tile_pool(name="ob", bufs=4) as obpool,
            tc.tile_pool(name="rl", bufs=16) as rlpool,
            tc.tile_pool(name="spsum", bufs=2, space="PSUM") as spsum,
            tc.tile_pool(name="rpsum", bufs=1, space="PSUM") as rpsum,
            tc.tile_pool(name="hpsum", bufs=2, space="PSUM") as hpsum,
            tc.tile_pool(name="opsum", bufs=1, space="PSUM") as opsum,
        ):
            # Resident constants — the s-chain deps (kh, then the chunk-0
            # xh below) are loaded first so the pipeline fills sooner; the
            # MLP constants follow (not needed until iteration 1).
            kh_t = cpool.tile([P, A], f32r)
            nc.sync.dma_start(out=kh_t[:], in_=kh_d[:])
            xh0_t = None  # placeholder, set right after pools
            x2_t = cpool.tile([P, NT, 4], f32)
            w1h_t = cpool.tile([P, H], bf16)
            b1p_t = cpool.tile([P, HC], f32)
            w2b_t = cpool.tile([P, HC * D_OUT], bf16)
            id_t = cpool.tile([P, P], bf16)
            m8_t = m8pool.tile([P, NT * 8], f32)
            # single-bank double-buffered w2 output accumulator
            o2_t = opsum.tile([P, 2, D_OUT], f32, name="o2")

            # PE warmup: back-to-back matmuls during pipeline fill push
            # the HAM clock gate to 8/8 (2.4 GHz); filler matmuls in the
            # first two iterations (below) keep it there until the real
            # w2 stream takes over.
            wu_t = cpool.tile([P, 512], bf16, name="warmup")
            nc.vector.memset(wu_t[:], 0.0)
            for r in range(24):
                nc.tensor.matmul(
                    out=o2_t[:, 0, :],
                    lhsT=wu_t[:, 0:P],
                    rhs=wu_t[:, 0:D_OUT],
                    start=True,
                    stop=True,
                )

            # Pipeline state carried across iterations
            pend_out = []      # deferred (+vg, DMA-out) work, 1 step behind
            xh_tiles = {}
            vg_tiles = {}      # chunk -> [8 tiles]
            rl_tiles = {}      # chunk -> [8 tiles]
            rlts_tiles = {}    # chunk -> [2 half tiles, SBUF]
            ht_tiles = {}      # chunk -> [2][HC] tiles
            idx_tiles = {}

            # Prefetch chunk 0 lhsT features, then the remaining consts
            xh_tiles[0] = xhpool.tile([P, TPC * P], f32r, tag="xh", name="xh0")
            nc.sync.dma_start(
                out=xh_tiles[0][:], in_=xh_d[:, 0 : TPC * P]
            )
            nc.sync.dma_start(out=x2_t[:], in_=x2_d[:])
            nc.sync.dma_start(out=w1h_t[:], in_=w1h_d[:])
            nc.sync.dma_start(out=b1p_t[:], in_=b1p_d[:])
            nc.sync.dma_start(out=w2b_t[:], in_=w2b_d[:])
            nc.sync.dma_start(out=id_t[:], in_=id_d[:])

            for i in range(NCHUNK + 2):
                cs, ch, cw = i, i - 1, i - 2

                if cs < NCHUNK:
                    idx_tiles[cs] = idxpool.tile([P, TPC, 8], u32, tag="idx", name=f"idxt{cs}")
                    vg_tiles[cs] = []
                    rl_tiles[cs] = []
                if 0 <= ch < NCHUNK:
                    ht_tiles[ch] = [
                        [
                            htpool.tile([P, 4, P], bf16,
                                        tag=f"ht{h}_{hc}",
                                        name=f"ht{ch}_{h}_{hc}")
                            for hc in range(HC)
                        ]
                        for h in range(2)
                    ]
                    # transpose rl -> rlt_ps [36, 512] (rows 0-3 and a
                    # replica at 32-35 for the second h row group), then one
                    # ACT copy per row-group to SBUF. half0 at iteration
                    # top; half1 after step 0 (gives the single rpsum bank
                    # time to drain through the ACT copies).
                    rlts_tiles[ch] = []

                    def emit_half_transpose(ch_, h_):
                        rlt_ps = rpsum.tile([4, 512], bf16, tag="rlt",
                                            name=f"rltps{ch_}_{h_}")
                        for q in range(4):
                            nc.tensor.transpose(
                                out=rlt_ps[:, q * P : (q + 1) * P],
                                in_=rl_tiles[ch_][h_ * 4 + q][:],
                                identity=id_t[:],
                            )
                        rlts = rltpool.tile([4, 512], bf16,
                                            tag=f"rlts{h_}",
                                            name=f"rlts{ch_}_{h_}")
                        nc.scalar.activation(
                            out=rlts[:],
                            in_=rlt_ps[:],
                            func=mybir.ActivationFunctionType.Copy,
                        )
                        rlts_tiles[ch_].append(rlts)

                    emit_half_transpose(ch, 0)

                for j in range(TPC):
                    # drain deferred output adds (their ACT copy is long
                    # done, so the GPS add never stalls the gather chain)
                    while len(pend_out) > 1:
                        cw_, j_, t_, ob_ = pend_out.pop(0)
                        nc.gpsimd.tensor_tensor(
                            out=ob_[:],
                            in0=ob_[:],
                            in1=vg_tiles[cw_][j_][:, 0:D_OUT],
                            op=mybir.AluOpType.add,
                        )
                        nc.sync.dma_start(
                            out=out_d[t_ * P : (t_ + 1) * P, :], in_=ob_[:]
                        )
                    if cs < NCHUNK:
                        t = cs * TPC + j
                        xh_t = xh_tiles[cs]
                        s_ps = spsum.tile([P, A], f32, tag="s")
                        for g in range(2):  # two 512-anchor halves, packed
                            nc.tensor.matmul(
                                out=s_ps[:, g * 512 : (g + 1) * 512],
                                lhsT=xh_t[
                                    32 * g : 32 * g + 5, j * P : (j + 1) * P
                                ],
                                rhs=kh_t[
                                    32 * g : 32 * g + 5, g * 512 : (g + 1) * 512
                                ],
                                start=True,
                                stop=True,
                                tile_position=(32 * g, 0),
                            )
                        m8 = m8_t[:, t * 8 : (t + 1) * 8]
                        nc.vector.max(m8, s_ps[:])
                        nc.vector.max_index(idx_tiles[cs][:, j, :], m8, s_ps[:])

                        if j == 0 and cs + 1 < NCHUNK:
                            xh_tiles[cs + 1] = xhpool.tile(
                                [P, TPC * P], f32r, tag="xh",
                                name=f"xh{cs + 1}",
                            )
                            nc.sync.dma_start(
                                out=xh_tiles[cs + 1][:],
                                in_=xh_d[
                                    :, (cs + 1) * TPC * P : (cs + 2) * TPC * P
                                ],
                            )

                        # Gather fused table row per token. NB: one offset per
                        # partition ([P,1]) per call into an offset-0 [P, W]
                        # dest tile — multi-index offsets and non-zero dest
                        # offsets are mishandled by the real SWDGE.
                        vg_j = vgpool.tile([P, VT_W], f32, tag=f"vg{j}")
                        nc.gpsimd.indirect_dma_start(
                            out=vg_j[:],
                            out_offset=None,
                            in_=vt_d[:],
                            in_offset=bass.IndirectOffsetOnAxis(
                                ap=idx_tiles[cs][:, j, 0:1], axis=0
                            ),
                        )
                        vg_tiles[cs].append(vg_j)

                        rl_j = rlpool.tile([P, 4], bf16, tag="rl",
                                           name=f"rl{cs}_{j}")
                        nc.gpsimd.tensor_tensor(
                            out=rl_j[:],
                            in0=x2_t[:, t, :],
                            in1=vg_j[:, D_OUT : D_OUT + 4],
                            op=mybir.AluOpType.subtract,
                        )
                        rl_tiles[cs].append(rl_j)

                        if j == TPC - 1:
                            nc.sync.dma_start(
                                out=idx_d[:, cs * TPC * 8 : (cs + 1) * TPC * 8],
                                in_=idx_tiles[cs][:],
                            )
                            nc.sync.dma_start(
                                out=m8_d[:, cs * TPC * 8 : (cs + 1) * TPC * 8],
                                in_=m8_t[:, cs * TPC * 8 : (cs + 1) * TPC * 8],
                            )

                    if 0 <= ch < NCHUNK and j == 1:
                        emit_half_transpose(ch, 1)

                    if 0 <= ch < NCHUNK:
                        # h-step: pair m=j%4 of half j//4 — two row groups
                        # compute hc=2m and hc=2m+1 concurrently into two
                        # different PSUM banks.
                        h, m = j // 4, j % 4
                        h_ps = [
                            hpsum.tile([P, 512], f32, tag="h",
                                       name=f"hps{ch}_{j}_{rg}")
                            for rg in range(2)
                        ]
                        for rg in range(2):
                            hc = 2 * m + rg
                            nc.tensor.matmul(
                                out=h_ps[rg][:],
                                lhsT=w1h_t[0:4, hc * P : (hc + 1) * P],
                                rhs=rlts_tiles[ch][h][:],
                                start=True,
                                stop=True,
                            )
                        for rg in range(2):
                            hc = 2 * m + rg
                            nc.scalar.activation(
                                out=ht_tiles[ch][h][hc][:],
                                in_=h_ps[rg][:],
                                func=mybir.ActivationFunctionType.Gelu,
                                bias=b1p_t[:, hc : hc + 1],
                            )

                    # w2 at 1.5-chunk skew: chunk c's half0 is consumed in
                    # steps 4-7 of iteration c+1 (its GELUs finished during
                    # steps 0-3); half1 in steps 0-3 of iteration c+2.
                    wt = None
                    if j < 4:
                        if cw >= 0:
                            wt = (cw, 1, j)
                    elif 0 <= ch < NCHUNK:
                        wt = (ch, 0, j - 4)
                    if wt is not None:
                        wc, wh, wq = wt
                        t = wc * TPC + wh * 4 + wq
                        o_ps = o2_t[:, j % 2, :]
                        for hc in range(HC):
                            nc.tensor.matmul(
                                out=o_ps,
                                lhsT=ht_tiles[wc][wh][hc][:, wq, :],
                                rhs=w2b_t[:, hc * D_OUT : (hc + 1) * D_OUT],
                                start=(hc == 0),
                                stop=(hc == HC - 1),
                            )
                        ob = obpool.tile([P, D_OUT], f32, tag="ob")
                        nc.scalar.activation(
                            out=ob[:],
                            in_=o_ps,
                            func=mybir.ActivationFunctionType.Copy,
                        )
                        pend_out.append((wc, wh * 4 + wq, t, ob))

                # drop references no longer needed
                if cw - 1 >= 0:
                    for dct in (vg_tiles, rl_tiles, rlts_tiles, ht_tiles,
                                idx_tiles, xh_tiles):
                        dct.pop(cw - 1, None)

            for cw_, j_, t_, ob_ in pend_out:
                nc.gpsimd.tensor_tensor(
                    out=ob_[:],
                    in0=ob_[:],
                    in1=vg_tiles[cw_][j_][:, 0:D_OUT],
                    op=mybir.AluOpType.add,
                )
                nc.sync.dma_start(
                    out=out_d[t_ * P : (t_ + 1) * P, :], in_=ob_[:]
                )
            pend_out.clear()

    nc.compile()
    names = ["xh", "kh", "x2", "w1h", "b1p", "w2b", "vt", "ident"]
    return nc, names


def _get_program():
    global _PROGRAM
    if _PROGRAM is None:
        _PROGRAM = _build_program()
    return _PROGRAM


def _host_pack(x, Key, init_mat, Value, w1, b1, w2, b2):
    """Build per-core input dicts (host-side layout packing)."""
    import ml_dtypes

    f = np.float32
    bf = ml_dtypes.bfloat16
    Key = np.asarray(Key, f)
    x = np.asarray(x, f)
    k2 = np.sum(Key * Key, axis=1)  # [A]

    # khat rows: [k0,k1,k2,1,|k|^2]; s = 2x.k - |x|^2 - |k|^2 = -d2
    kh = np.zeros((P, A), f)
    kf = np.concatenate([Key, np.ones((A, 1), f), k2[:, None]], axis=1)  # [A,5]
    for g in range(2):
        kh[32 * g : 32 * g + 5, :] = kf.T

    # w1h rows 32g..32g+2 = 0.5*w1 (replicated for 4 row groups), row 3 = 0
    w1h = np.zeros((P, H), f)
    for g in range(4):
        w1h[32 * g : 32 * g + 3, :] = 0.5 * np.asarray(w1, f)
    w1h = w1h.astype(bf)

    b1p = np.asarray(b1, f).reshape(H // P, P).T.copy()  # [128, 8]
    w2b = (
        np.asarray(w2, f)
        .reshape(H // P, P, D_OUT)
        .transpose(1, 0, 2)
        .reshape(P, (H // P) * D_OUT)
        .astype(bf)
    )
    vt = np.zeros((A, VT_W), f)
    vt[:, :D_OUT] = np.asarray(init_mat, f) + np.asarray(Value, f) + np.asarray(b2, f)
    vt[:, D_OUT : D_OUT + 3] = 2.0 * Key
    ident = np.eye(P, dtype=f).astype(bf)

    in_maps = []
    for c in range(N_CORES):
        xc = x[c]  # [N, 3]
        x2sq = np.sum(xc * xc, axis=1)  # [N]
        # xhat features [N, 5]: [2x, -|x|^2, -1]
        xf = np.concatenate(
            [2.0 * xc, -x2sq[:, None], -np.ones((N, 1), f)], axis=1
        ).astype(f)
        # packed lhsT [128, N]: tile t at cols t*128..; features at rows 0-4
        # plus a replica at rows 32-36 for the second row-group.
        xh = np.zeros((P, N), f)
        xf_t = xf.reshape(NT, P, 5).transpose(2, 0, 1).reshape(5, N)  # [5, NT*P]
        xh[0:5, :] = xf_t
        xh[32:37, :] = xf_t

        x2q = np.zeros((N, 4), f)
        x2q[:, :3] = 2.0 * xc
        x2 = x2q.reshape(NT, P, 4).transpose(1, 0, 2).reshape(P, NT * 4).copy()

        in_maps.append(
            {
                "xh": xh,
                "kh": kh,
                "x2": x2,
                "w1h": w1h,
                "b1p": b1p,
                "w2b": w2b,
                "vt": vt,
                "ident": ident,
            }
        )
    return in_maps


def _erf(z):
    # Abramowitz-Stegun is not enough; use the exact erf from scipy if
    # present, else jax (available wherever the bass stack runs).
    try:
        from scipy.special import erf

        return erf(z)
    except ImportError:
        import jax

        with jax.default_device(jax.devices("cpu")[0]):
            return np.asarray(jax.scipy.special.erf(np.asarray(z, np.float32)))


def _refine(out, m8o, idxo, x, Key, init_mat, Value, w1, b1, w2, b2, tau=0.03):
    """Re-resolve tokens whose top-2 score gap is within tau (near-ties):
    recompute their argmin + output row in exact fp32 reference arithmetic."""
    f = np.float32
    Key = np.asarray(Key, f)
    V = np.asarray(init_mat, f) + np.asarray(Value, f)
    k2 = np.sum(Key * Key, axis=1)
    n_fixed = 0
    for c in range(out.shape[0]):
        m8 = m8o[c]  # [128, NT*8]
        m0 = m8[:, 0::8]  # [128, NT]
        m1 = m8[:, 1::8]
        gap = m0 - m1  # s-space gap == d2 second - d2 min
        dev_idx = idxo[c][:, 0::8].astype(np.int64)  # [128, NT]
        scale = 1.0 + np.abs(m0)
        flag = gap < tau * scale  # [128, NT]
        ps, ts = np.nonzero(flag)
        if ps.size == 0:
            continue
        toks = ts * P + ps
        xc = np.asarray(x[c], f)[toks]  # [F, 3]
        d2 = -2.0 * (xc @ Key.T) + k2[None, :]  # reference formula, fp32
        amin = np.argmin(d2, axis=1)
        mism = amin != dev_idx[ps, ts]
        if not np.any(mism):
            continue
        toks = toks[mism]
        amin = amin[mism]
        xe = np.asarray(x[c], f)[toks]
        rl = xe - Key[amin]
        pre = (rl @ np.asarray(w1, f) + np.asarray(b1, f)).astype(f)
        h = (0.5 * pre * (1.0 + _erf(pre / np.sqrt(f(2.0))))).astype(f)
        row = (h @ np.asarray(w2, f) + np.asarray(b2, f) + V[amin]).astype(f)
        out[c, toks, :] = row
        n_fixed += toks.size
    return n_fixed


def kernel(**inputs):
    from concourse.bass_utils import run_bass_kernel_spmd

    nc, names = _get_program()
    in_maps = _host_pack(**inputs)
    res = run_bass_kernel_spmd(nc, in_maps, core_ids=list(range(N_CORES)))

    out = np.zeros((B, N, D_OUT), np.float32)
    m8o = np.zeros((B, P, NT * 8), np.float32)
    idxo = np.zeros((B, P, NT * 8), np.uint32)
    for c in range(N_CORES):
        r = res.results[c]
        out[c] = r["outp"]
        m8o[c] = r["m8o"]
        idxo[c] = r["idxo"]

    _refine(out, m8o, idxo, **inputs)
    return out


if __name__ == "__main__":
    # smoke: build only
    _get_program()
    print("program built")


# revision 38
# speedup vs baseline: 1.0377x; 1.0265x over previous
"""FAISS-anchor kernel layer on 8 Trainium2 NeuronCores (Bass/Tile).

Problem (per full input):
    x [8,8192,3], Key [1024,3], init_mat/Value [1024,256],
    w1 [3,1024], b1 [1024], w2 [1024,256], b2 [256]
    idx = argmin_a ||x - Key_a||^2           (exact 1-NN, first-tie)
    out = gelu((x - Key[idx]) @ w1 + b1) @ w2 + b2 + (init_mat + Value)[idx]

Sharding: pure data-parallel — core c takes batch element c (8192 tokens).
All tables (Key-derived features, V-table, MLP weights) are replicated.

Device pipeline, software-pipelined with a 2-chunk skew so the PE stream
never head-of-line blocks on the gather chain. Per tile-step j of
iteration i (chunks cs=i, ch=i-1, cw=i-2):
    PE:   s(cs,j)  = -d2+|x|^2 for 1024 anchors (two row-group-packed
          f32r matmuls, K=5 feature folding)
    DVE:  max8 -> top-8; max_index -> anchor idx (first-tie == argmin)
    DMA:  indirect gather of fused row [V+init+b2 | 2*Key] per token
    GPS:  rl = 2x - 2*Key[idx]  (bf16 out)
    PE:   transpose rl -> [4,tok] bf16 at row 0 and 32 (iteration top);
          h-step: 2-way row-group-packed bf16 matmuls (K=4), the two
          groups compute different hc -> different PSUM banks
    ACT:  gelu(h + b1) -> ht bf16
    PE:   w2-step(cw,q=j): out = ht_q @ w2 (bf16, K=8x128 accum)
    ACT+GPS: PSUM->SBUF copy, += gathered V row; DMA out.

Host: packs layouts, runs 8 cores via run_bass_kernel_spmd, re-assembles,
and re-resolves near-tie tokens (top-2 gap below tau) with exact fp32
reference arithmetic so matmul rounding cannot flip the argmin.
"""

import numpy as np

B, N, A, D_IN, D_OUT = 8, 8192, 1024, 3, 256
H = 4 * D_OUT
P = 128
NT = N // P            # 64 token tiles per core
TPC = 8                # tiles per chunk
NCHUNK = NT // TPC     # 8 chunks
VT_W = 272             # gather-table row width (256 V + 3 key + pad), 1088B
N_CORES = 8
HC = H // P            # 8 H-chunks

_PROGRAM = None  # (nc, input_names)


def _build_program():
    import concourse.bass as bass
    import concourse.mybir as mybir
    import concourse.tile as tile
    from concourse import bacc

    f32 = mybir.dt.float32
    f32r = mybir.dt.float32r
    bf16 = mybir.dt.bfloat16
    u32 = mybir.dt.uint32

    # Bacc (not raw Bass): its compile() splits multi-sem waits and moves
    # matmul waits onto ldweights — TRN2 allows at most 1 wait per instr.
    nc = bacc.Bacc("TRN2", target_bir_lowering=False, debug=False)

    # DRAM I/O
    xh_d = nc.dram_tensor("xh", [P, N], f32r, kind="ExternalInput").ap()
    kh_d = nc.dram_tensor("kh", [P, A], f32r, kind="ExternalInput").ap()
    x2_d = nc.dram_tensor("x2", [P, NT * 4], f32, kind="ExternalInput").ap()
    w1h_d = nc.dram_tensor("w1h", [P, H], bf16, kind="ExternalInput").ap()
    b1p_d = nc.dram_tensor("b1p", [P, HC], f32, kind="ExternalInput").ap()
    w2b_d = nc.dram_tensor("w2b", [P, HC * D_OUT], bf16, kind="ExternalInput").ap()
    vt_d = nc.dram_tensor("vt", [A, VT_W], f32, kind="ExternalInput").ap()
    id_d = nc.dram_tensor("ident", [P, P], bf16, kind="ExternalInput").ap()

    out_d = nc.dram_tensor("outp", [N, D_OUT], f32, kind="ExternalOutput").ap()
    m8_d = nc.dram_tensor("m8o", [P, NT * 8], f32, kind="ExternalOutput").ap()
    idx_d = nc.dram_tensor("idxo", [P, NT * 8], u32, kind="ExternalOutput").ap()

    with tile.TileContext(nc) as tc:
        with (
            tc.tile_pool(name="const", bufs=1) as cpool,
            tc.tile_pool(name="xh", bufs=2) as xhpool,
            tc.tile_pool(name="vg", bufs=4) as vgpool,
            tc.tile_pool(name="rlt", bufs=2) as rltpool,
            tc.tile_pool(name="ht", bufs=2) as htpool,
            tc.tile_pool(name="m8", bufs=1) as m8pool,
            tc.tile_pool(name="idx", bufs=2) as idxpool,
            tc.tile_pool(name="ob", bufs=4) as obpool,
            tc.tile_pool(name="rl", bufs=16) as rlpool,
            tc.tile_pool(name="spsum", bufs=2, space="PSUM") as spsum,
            tc.tile_pool(name="rpsum", bufs=1, space="PSUM") as rpsum,
            tc.tile_pool(name="hpsum", bufs=2, space="PSUM") as hpsum,
            tc.tile_pool(name="opsum", bufs=1, space="PSUM") as opsum,
        ):
            # Resident constants — the s-chain deps (kh, then the chunk-0
            # xh below) are loaded first so the pipeline fills sooner; the
            # MLP constants follow (not needed until iteration 1).
            kh_t = cpool.tile([P, A], f32r)
            nc.sync.dma_start(out=kh_t[:], in_=kh_d[:])
            xh0_t = None  # placeholder, set right after pools
            x2_t = cpool.tile([P, NT, 4], f32)
            w1h_t = cpool.tile([P, H], bf16)
            b1p_t = cpool.tile([P, HC], f32)
            w2b_t = cpool.tile([P, HC * D_OUT], bf16)
            id_t = cpool.tile([P, P], bf16)
            m8_t = m8pool.tile([P, NT * 8], f32)
            # single-bank double-buffered w2 output accumulator
            o2_t = opsum.tile([P, 2, D_OUT], f32, name="o2")

            # PE warmup: back-to-back matmuls during pipeline fill push
            # the HAM clock gate to 8/8 (2.4 GHz); filler matmuls in the
            # first two iterations (below) keep it there until the real
            # w2 stream takes over.
            wu_t = cpool.tile([P, 512], bf16, name="warmup")
            nc.vector.memset(wu_t[:], 0.0)
            for r in range(24):
                nc.tensor.matmul(
                    out=o2_t[:, 0, :],
                    lhsT=wu_t[:, 0:P],
                    rhs=wu_t[:, 0:D_OUT],
                    start=True,
                    stop=True,
                )

            # Pipeline state carried across iterations
            pend_out = []      # deferred (+vg, DMA-out) work, 1 step behind
            xh_tiles = {}
            vg_tiles = {}      # chunk -> [8 tiles]
            rl_tiles = {}      # chunk -> [8 tiles]
            rlts_tiles = {}    # chunk -> [2 half tiles, SBUF]
            ht_tiles = {}      # chunk -> [2][HC] tiles
            idx_tiles = {}

            # Prefetch chunk 0 lhsT features, then the remaining consts
            xh_tiles[0] = xhpool.tile([P, TPC * P], f32r, tag="xh", name="xh0")
            nc.sync.dma_start(
                out=xh_tiles[0][:], in_=xh_d[:, 0 : TPC * P]
            )
            nc.sync.dma_start(out=x2_t[:], in_=x2_d[:])
            nc.sync.dma_start(out=w1h_t[:], in_=w1h_d[:])
            nc.sync.dma_start(out=b1p_t[:], in_=b1p_d[:])
            nc.sync.dma_start(out=w2b_t[:], in_=w2b_d[:])
            nc.sync.dma_start(out=id_t[:], in_=id_d[:])

            def emit_half_transpose(ch_, h_):
                rlt_ps = rpsum.tile([4, 512], bf16, tag="rlt",
                                    name=f"rltps{ch_}_{h_}")
                for q in range(4):
                    nc.tensor.transpose(
                        out=rlt_ps[:, q * P : (q + 1) * P],
                        in_=rl_tiles[ch_][h_ * 4 + q][:],
                        identity=id_t[:],
                    )
                rlts = rltpool.tile([4, 512], bf16,
                                    tag=f"rlts{h_}",
                                    name=f"rlts{ch_}_{h_}")
                nc.scalar.activation(
                    out=rlts[:],
                    in_=rlt_ps[:],
                    func=mybir.ActivationFunctionType.Copy,
                )
                rlts_tiles[ch_].append(rlts)

            for i in range(NCHUNK + 2):
                cs, ch, cw = i, i - 1, i - 2

                if cs < NCHUNK:
                    idx_tiles[cs] = idxpool.tile([P, TPC, 8], u32, tag="idx", name=f"idxt{cs}")
                    vg_tiles[cs] = []
                    rl_tiles[cs] = []
                    rlts_tiles[cs] = []
                if 0 <= ch < NCHUNK:
                    ht_tiles[ch] = [
                        [
                            htpool.tile([P, 4, P], bf16,
                                        tag=f"ht{h}_{hc}",
                                        name=f"ht{ch}_{h}_{hc}")
                            for hc in range(HC)
                        ]
                        for h in range(2)
                    ]
                    # half1's transposes run at iteration top (its h-steps
                    # only start at step 4); half0's ran at step 5 of its own
                    # s-iteration, so its rlts are ready before step 0 here.
                    emit_half_transpose(ch, 1)

                for j in range(TPC):
                    # drain deferred output adds (their ACT copy is long
                    # done, so the GPS add never stalls the gather chain)
                    while len(pend_out) > 1:
                        cw_, j_, t_, ob_ = pend_out.pop(0)
                        nc.gpsimd.tensor_tensor(
                            out=ob_[:],
                            in0=ob_[:],
                            in1=vg_tiles[cw_][j_][:, 0:D_OUT],
                            op=mybir.AluOpType.add,
                        )
                        nc.sync.dma_start(
                            out=out_d[t_ * P : (t_ + 1) * P, :], in_=ob_[:]
                        )
                    if cs < NCHUNK:
                        t = cs * TPC + j
                        xh_t = xh_tiles[cs]
                        s_ps = spsum.tile([P, A], f32, tag="s")
                        for g in range(2):  # two 512-anchor halves, packed
                            nc.tensor.matmul(
                                out=s_ps[:, g * 512 : (g + 1) * 512],
                                lhsT=xh_t[
                                    32 * g : 32 * g + 5, j * P : (j + 1) * P
                                ],
                                rhs=kh_t[
                                    32 * g : 32 * g + 5, g * 512 : (g + 1) * 512
                                ],
                                start=True,
                                stop=True,
                                tile_position=(32 * g, 0),
                            )
                        m8 = m8_t[:, t * 8 : (t + 1) * 8]
                        nc.vector.max(m8, s_ps[:])
                        nc.vector.max_index(idx_tiles[cs][:, j, :], m8, s_ps[:])

                        if j == 0 and cs + 1 < NCHUNK:
                            xh_tiles[cs + 1] = xhpool.tile(
                                [P, TPC * P], f32r, tag="xh",
                                name=f"xh{cs + 1}",
                            )
                            nc.sync.dma_start(
                                out=xh_tiles[cs + 1][:],
                                in_=xh_d[
                                    :, (cs + 1) * TPC * P : (cs + 2) * TPC * P
                                ],
                            )

                        # Gather fused table row per token. NB: one offset per
                        # partition ([P,1]) per call into an offset-0 [P, W]
                        # dest tile — multi-index offsets and non-zero dest
                        # offsets are mishandled by the real SWDGE.
                        vg_j = vgpool.tile([P, VT_W], f32, tag=f"vg{j}")
                        nc.gpsimd.indirect_dma_start(
                            out=vg_j[:],
                            out_offset=None,
                            in_=vt_d[:],
                            in_offset=bass.IndirectOffsetOnAxis(
                                ap=idx_tiles[cs][:, j, 0:1], axis=0
                            ),
                        )
                        vg_tiles[cs].append(vg_j)

                        rl_j = rlpool.tile([P, 4], bf16, tag="rl",
                                           name=f"rl{cs}_{j}")
                        nc.gpsimd.tensor_tensor(
                            out=rl_j[:],
                            in0=x2_t[:, t, :],
                            in1=vg_j[:, D_OUT : D_OUT + 4],
                            op=mybir.AluOpType.subtract,
                        )
                        rl_tiles[cs].append(rl_j)

                        if j == TPC - 1:
                            nc.sync.dma_start(
                                out=idx_d[:, cs * TPC * 8 : (cs + 1) * TPC * 8],
                                in_=idx_tiles[cs][:],
                            )
                            nc.sync.dma_start(
                                out=m8_d[:, cs * TPC * 8 : (cs + 1) * TPC * 8],
                                in_=m8_t[:, cs * TPC * 8 : (cs + 1) * TPC * 8],
                            )
                        if j == 5:
                            emit_half_transpose(cs, 0)

                    if 0 <= ch < NCHUNK:
                        # h-step: pair m=j%4 of half j//4 — two row groups
                        # compute hc=2m and hc=2m+1 concurrently into two
                        # different PSUM banks.
                        h, m = j // 4, j % 4
                        h_ps = [
                            hpsum.tile([P, 512], f32, tag="h",
                                       name=f"hps{ch}_{j}_{rg}")
                            for rg in range(2)
                        ]
                        for rg in range(2):
                            hc = 2 * m + rg
                            nc.tensor.matmul(
                                out=h_ps[rg][:],
                                lhsT=w1h_t[0:4, hc * P : (hc + 1) * P],
                                rhs=rlts_tiles[ch][h][:],
                                start=True,
                                stop=True,
                            )
                        for rg in range(2):
                            hc = 2 * m + rg
                            nc.scalar.activation(
                                out=ht_tiles[ch][h][hc][:],
                                in_=h_ps[rg][:],
                                func=mybir.ActivationFunctionType.Gelu,
                                bias=b1p_t[:, hc : hc + 1],
                            )

                    # w2 at 1.5-chunk skew: chunk c's half0 is consumed in
                    # steps 4-7 of iteration c+1 (its GELUs finished during
                    # steps 0-3); half1 in steps 0-3 of iteration c+2.
                    wt = None
                    if j < 4:
                        if cw >= 0:
                            wt = (cw, 1, j)
                    elif 0 <= ch < NCHUNK:
                        wt = (ch, 0, j - 4)
                    if wt is not None:
                        wc, wh, wq = wt
                        t = wc * TPC + wh * 4 + wq
                        o_ps = o2_t[:, j % 2, :]
                        for hc in range(HC):
                            nc.tensor.matmul(
                                out=o_ps,
                                lhsT=ht_tiles[wc][wh][hc][:, wq, :],
                                rhs=w2b_t[:, hc * D_OUT : (hc + 1) * D_OUT],
                                start=(hc == 0),
                                stop=(hc == HC - 1),
                            )
                        ob = obpool.tile([P, D_OUT], f32, tag="ob")
                        nc.scalar.activation(
                            out=ob[:],
                            in_=o_ps,
                            func=mybir.ActivationFunctionType.Copy,
                        )
                        pend_out.append((wc, wh * 4 + wq, t, ob))

                # drop references no longer needed
                if cw - 1 >= 0:
                    for dct in (vg_tiles, rl_tiles, rlts_tiles, ht_tiles,
                                idx_tiles, xh_tiles):
                        dct.pop(cw - 1, None)

            for cw_, j_, t_, ob_ in pend_out:
                nc.gpsimd.tensor_tensor(
                    out=ob_[:],
                    in0=ob_[:],
                    in1=vg_tiles[cw_][j_][:, 0:D_OUT],
                    op=mybir.AluOpType.add,
                )
                nc.sync.dma_start(
                    out=out_d[t_ * P : (t_ + 1) * P, :], in_=ob_[:]
                )
            pend_out.clear()

    nc.compile()
    names = ["xh", "kh", "x2", "w1h", "b1p", "w2b", "vt", "ident"]
    return nc, names


def _get_program():
    global _PROGRAM
    if _PROGRAM is None:
        _PROGRAM = _build_program()
    return _PROGRAM


def _host_pack(x, Key, init_mat, Value, w1, b1, w2, b2):
    """Build per-core input dicts (host-side layout packing)."""
    import ml_dtypes

    f = np.float32
    bf = ml_dtypes.bfloat16
    Key = np.asarray(Key, f)
    x = np.asarray(x, f)
    k2 = np.sum(Key * Key, axis=1)  # [A]

    # khat rows: [k0,k1,k2,1,|k|^2]; s = 2x.k - |x|^2 - |k|^2 = -d2
    kh = np.zeros((P, A), f)
    kf = np.concatenate([Key, np.ones((A, 1), f), k2[:, None]], axis=1)  # [A,5]
    for g in range(2):
        kh[32 * g : 32 * g + 5, :] = kf.T

    # w1h rows 32g..32g+2 = 0.5*w1 (replicated for 4 row groups), row 3 = 0
    w1h = np.zeros((P, H), f)
    for g in range(4):
        w1h[32 * g : 32 * g + 3, :] = 0.5 * np.asarray(w1, f)
    w1h = w1h.astype(bf)

    b1p = np.asarray(b1, f).reshape(H // P, P).T.copy()  # [128, 8]
    w2b = (
        np.asarray(w2, f)
        .reshape(H // P, P, D_OUT)
        .transpose(1, 0, 2)
        .reshape(P, (H // P) * D_OUT)
        .astype(bf)
    )
    vt = np.zeros((A, VT_W), f)
    vt[:, :D_OUT] = np.asarray(init_mat, f) + np.asarray(Value, f) + np.asarray(b2, f)
    vt[:, D_OUT : D_OUT + 3] = 2.0 * Key
    ident = np.eye(P, dtype=f).astype(bf)

    in_maps = []
    for c in range(N_CORES):
        xc = x[c]  # [N, 3]
        x2sq = np.sum(xc * xc, axis=1)  # [N]
        # xhat features [N, 5]: [2x, -|x|^2, -1]
        xf = np.concatenate(
            [2.0 * xc, -x2sq[:, None], -np.ones((N, 1), f)], axis=1
        ).astype(f)
        # packed lhsT [128, N]: tile t at cols t*128..; features at rows 0-4
        # plus a replica at rows 32-36 for the second row-group.
        xh = np.zeros((P, N), f)
        xf_t = xf.reshape(NT, P, 5).transpose(2, 0, 1).reshape(5, N)  # [5, NT*P]
        xh[0:5, :] = xf_t
        xh[32:37, :] = xf_t

        x2q = np.zeros((N, 4), f)
        x2q[:, :3] = 2.0 * xc
        x2 = x2q.reshape(NT, P, 4).transpose(1, 0, 2).reshape(P, NT * 4).copy()

        in_maps.append(
            {
                "xh": xh,
                "kh": kh,
                "x2": x2,
                "w1h": w1h,
                "b1p": b1p,
                "w2b": w2b,
                "vt": vt,
                "ident": ident,
            }
        )
    return in_maps


def _erf(z):
    # Abramowitz-Stegun is not enough; use the exact erf from scipy if
    # present, else jax (available wherever the bass stack runs).
    try:
        from scipy.special import erf

        return erf(z)
    except ImportError:
        import jax

        with jax.default_device(jax.devices("cpu")[0]):
            return np.asarray(jax.scipy.special.erf(np.asarray(z, np.float32)))


def _refine(out, m8o, idxo, x, Key, init_mat, Value, w1, b1, w2, b2, tau=0.03):
    """Re-resolve tokens whose top-2 score gap is within tau (near-ties):
    recompute their argmin + output row in exact fp32 reference arithmetic."""
    f = np.float32
    Key = np.asarray(Key, f)
    V = np.asarray(init_mat, f) + np.asarray(Value, f)
    k2 = np.sum(Key * Key, axis=1)
    n_fixed = 0
    for c in range(out.shape[0]):
        m8 = m8o[c]  # [128, NT*8]
        m0 = m8[:, 0::8]  # [128, NT]
        m1 = m8[:, 1::8]
        gap = m0 - m1  # s-space gap == d2 second - d2 min
        dev_idx = idxo[c][:, 0::8].astype(np.int64)  # [128, NT]
        scale = 1.0 + np.abs(m0)
        flag = gap < tau * scale  # [128, NT]
        ps, ts = np.nonzero(flag)
        if ps.size == 0:
            continue
        toks = ts * P + ps
        xc = np.asarray(x[c], f)[toks]  # [F, 3]
        d2 = -2.0 * (xc @ Key.T) + k2[None, :]  # reference formula, fp32
        amin = np.argmin(d2, axis=1)
        mism = amin != dev_idx[ps, ts]
        if not np.any(mism):
            continue
        toks = toks[mism]
        amin = amin[mism]
        xe = np.asarray(x[c], f)[toks]
        rl = xe - Key[amin]
        pre = (rl @ np.asarray(w1, f) + np.asarray(b1, f)).astype(f)
        h = (0.5 * pre * (1.0 + _erf(pre / np.sqrt(f(2.0))))).astype(f)
        row = (h @ np.asarray(w2, f) + np.asarray(b2, f) + V[amin]).astype(f)
        out[c, toks, :] = row
        n_fixed += toks.size
    return n_fixed


def kernel(**inputs):
    from concourse.bass_utils import run_bass_kernel_spmd

    nc, names = _get_program()
    in_maps = _host_pack(**inputs)
    res = run_bass_kernel_spmd(nc, in_maps, core_ids=list(range(N_CORES)))

    out = np.zeros((B, N, D_OUT), np.float32)
    m8o = np.zeros((B, P, NT * 8), np.float32)
    idxo = np.zeros((B, P, NT * 8), np.uint32)
    for c in range(N_CORES):
        r = res.results[c]
        out[c] = r["outp"]
        m8o[c] = r["m8o"]
        idxo[c] = r["idxo"]

    _refine(out, m8o, idxo, **inputs)
    return out


if __name__ == "__main__":
    # smoke: build only
    _get_program()
    print("program built")
